# revision 50
# baseline (speedup 1.0000x reference)
"""Chamfer + edge + normal-cosine combined loss on 8 Trainium2 cores.

Candidate-pruned distance scan. The host kd-sorts both point sets per batch,
computes bbox lower bounds LB(64-pt t-tile, 8-pt p-group) and per-point
achievable upper bounds on nearest distances, and keeps only (tile, group)
pairs that can contain a row-min (LB <= UB_tile) or a column-min
(LB <= UB_group) -- provably covering every exact row/column argmin (~10-20%
of all pairs survive). Kept groups are packed into 512-column chunks and
dealt evenly to the 8 cores (any core may process any batch's chunks).

The device is a pure streaming scanner. Per step, eight K=13 matmuls (2-way
bf16-split factors; tile_position row-groups 32j x col-groups 64k) compute
eight [64, 512] chunks of M = -P into four [128, 512] PSUM tiles; ACT casts
two of them to fp8 staging while DVE casts the other two in parallel (fp8
halves the ship bytes; rounding is monotone so the true row/col argmax always
wins or ties), and per-step DMAs on the gpsimd/sync queues ship the chunks to
DRAM while sync/gpsimd/scalar queues stream the next inputs in.

Host finish: fp8 row/col maxes per tile/column, ALL tying candidates
recomputed exactly in fp64 (restores ~1e-7 accuracy), then the tiny edge and
normal-cosine terms in numpy as in the reference.
"""

from contextlib import ExitStack

import ml_dtypes
import numpy as np

B = 4
N = 8192
NCORES = 8
TIL = 64             # t rows per tile (two tiles stack in 128 partitions)
GRP = 4              # fine p-group size for pruning
CHW = 512            # chunk width in columns
GPC = CHW // GRP     # 32 fine groups per chunk
NT = N // TIL        # 64 t-tiles per batch
NGR = N // GRP       # 512 fine groups per batch
KS = 13              # bf16 split rows (2-way split: ~1e-4 abs error on P,
                     # absorbed by the host's exact tie resolution)
KSP = 16             # split rows padded for DMA blocks
NNEAR = 5            # groups sampled for upper bounds

_LAST_RESULTS = {}


# ---------------------------------------------------------------- host: split
def _split3(x):
    h = x.astype(ml_dtypes.bfloat16)
    r1 = x - h.astype(np.float32)
    m = r1.astype(ml_dtypes.bfloat16)
    r2 = r1 - m.astype(np.float32)
    l = r2.astype(ml_dtypes.bfloat16)
    return h, m, l


def _build_split_rows(L, R):
    """L [5, X], R [5, Y] fp32 term rows -> bf16 [13, X], [13, Y].

    M = sum_k L[k] (outer) R[k] = 2<g,p> - |g|^2 - |p|^2 = -P; 2-way bf16
    split keeps products accurate to ~2^-16 relative (~1e-4 abs on P)."""
    outL, outR = [], []
    for c in range(3):
        Lh, Lm, _ = _split3(L[c])
        Rh, Rm, _ = _split3(R[c])
        for a, b in ((Lh, Rh), (Lh, Rm), (Lm, Rh)):
            outL.append(a)
            outR.append(b)
    Xh, Xm, _ = _split3(L[3])
    negone = R[3].astype(ml_dtypes.bfloat16)
    for a in (Xh, Xm):
        outL.append(a)
        outR.append(negone)
    Yh, Ym, _ = _split3(R[4])
    one = L[4].astype(ml_dtypes.bfloat16)
    for b in (Yh, Ym):
        outL.append(one)
        outR.append(b)
    return np.ascontiguousarray(np.stack(outL)), np.ascontiguousarray(np.stack(outR))


# -------------------------------------------------------------- host: pruning
def _kd_order(pts, leaf):
    """Balanced kd-tree order: median split on widest axis down to `leaf`."""
    out = []
    stack = [np.arange(len(pts))]
    while stack:
        ids = stack.pop()
        if len(ids) <= leaf:
            out.append(ids)
            continue
        p = pts[ids]
        ax = int((p.max(0) - p.min(0)).argmax())
        k = len(ids) // 2
        o = np.argpartition(p[:, ax], k)
        stack.append(ids[o[k:]])
        stack.append(ids[o[:k]])
    # stack order: first-pushed-last; rebuild in left-to-right order
    return np.concatenate(out)


def _point_ubs(A, Btiles, nnear):
    """For each point in A [n,3]: an achievable nearest-distance^2 upper bound,
    the min over all points of the `nnear` nearest B-tiles (by center)."""
    bc = Btiles.mean(1)
    d = ((A[:, None, :] - bc[None, :, :]) ** 2).sum(-1)
    near = np.argpartition(d, nnear, axis=1)[:, :nnear]
    ub = np.full(len(A), np.inf)
    for j in range(nnear):
        sel = near[:, j]
        for g in np.unique(sel):
            m = sel == g
            dd = ((A[m][:, None, :] - Btiles[g][None, :, :]) ** 2).sum(-1).min(1)
            ub[m] = np.minimum(ub[m], dd)
    return ub


def _prep_batch(preds_b, gts_b):
    """Returns sorted perms, per-tile candidate chunk lists and split tables."""
    po = _kd_order(preds_b, GRP)
    go = _kd_order(gts_b, TIL)
    Ps = preds_b[po].astype(np.float64)
    Gs = gts_b[go].astype(np.float64)

    Pt = Ps.reshape(NGR, GRP, 3)
    Gt = Gs.reshape(NT, TIL, 3)
    plo, phi = Pt.min(1), Pt.max(1)
    glo, ghi = Gt.min(1), Gt.max(1)
    d1 = np.maximum(0.0, plo[None, :, :] - ghi[:, None, :])
    d2 = np.maximum(0.0, glo[:, None, :] - phi[None, :, :])
    LB = (np.maximum(d1, d2) ** 2).sum(-1)           # [NT, NGR]

    ub_t = _point_ubs(Gs, Pt, NNEAR)
    UB_T = ub_t.reshape(NT, TIL).max(1)              # [NT]
    ub_p = _point_ubs(Ps, Gt, NNEAR)
    UB_G = ub_p.reshape(NGR, GRP).max(1)             # [NGR]

    keep = LB <= np.maximum(UB_T[:, None], UB_G[None, :]) * (1.0 + 1e-6) + 1e-12

    # chunk lists: per tile, its fine groups packed into CHW-wide chunks
    chunks = []                                      # (tile, group_ids[GPC])
    for T in range(NT):
        gl = np.nonzero(keep[T])[0]
        padded = ((len(gl) + GPC - 1) // GPC) * GPC
        gl = np.resize(gl, padded)      # cycles values to pad
        for c in range(len(gl) // GPC):
            chunks.append((T, gl[c * GPC:(c + 1) * GPC]))

    # split tables over sorted points
    xsq = (Gs * Gs).sum(-1).astype(np.float32)
    ysq = (Ps * Ps).sum(-1).astype(np.float32)
    L = np.empty((5, N), np.float32)
    L[0:3] = (2.0 * Gs.T).astype(np.float32)
    L[3] = xsq
    L[4] = 1.0
    R = np.empty((5, N), np.float32)
    R[0:3] = Ps.T.astype(np.float32)
    R[3] = -1.0
    R[4] = -ysq
    sL, sR = _build_split_rows(L, R)                 # [24, N] bf16 each
    return dict(po=po, go=go, chunks=chunks, sL=sL, sR=sR)


def _prep(preds, gts):
    metas = [_prep_batch(preds[b], gts[b]) for b in range(B)]
    # global load balance: flatten (batch, tile, groups) chunks, deal to cores
    allch = []
    for b in range(B):
        allch.extend((b,) + ch for ch in metas[b]['chunks'])
    per = (len(allch) + NCORES - 1) // NCORES
    raw = [allch[c * per:(c + 1) * per] for c in range(NCORES)]
    nreal = [len(cc) for cc in raw]
    # 8 chunks per step (4 row-groups x 2 col-groups); 2 steps per in-DMA
    steps = max((n + 7) // 8 for n in nreal)
    steps += steps % 2
    in_maps = []
    core_data = []
    for c in range(NCORES):
        cc = list(raw[c]) or [allch[0]]
        while len(cc) < steps * 8:
            cc.append(cc[-1])
        # packed row blocks: in0[s2, j] = [16, 2304] for j-block partitions
        # 32j+[0,16); cols ph*1152 + k*576 + [0:64 weights | 64:576 rhs]
        in0 = np.zeros((steps // 2, 4, KSP, 2304), ml_dtypes.bfloat16)
        colmap = np.empty((steps * 8, CHW), np.int32)
        tileof = np.empty(steps * 8, np.int32)
        batof = np.empty(steps * 8, np.int32)
        for i, (bb, T, gl) in enumerate(cc):
            sL, sR = metas[bb]['sL'], metas[bb]['sR']
            cols = (gl[:, None] * GRP + np.arange(GRP)[None, :]).ravel()
            s, slot = divmod(i, 8)
            k, j = divmod(slot, 4)
            s2, ph = divmod(s, 2)
            o = ph * 1152 + k * 576
            in0[s2, j, 0:KS, o:o + 64] = sL[:, T * TIL:(T + 1) * TIL]
            in0[s2, j, 0:KS, o + 64:o + 576] = sR[:, cols]
            colmap[i] = cols
            tileof[i] = T
            batof[i] = bb
        in_maps.append({"in0": in0})
        core_data.append((cc, colmap, tileof, batof))
    return metas, core_data, nreal, steps, in_maps


# ------------------------------------------------------------------- device
def _build_nc(steps):
    import concourse.mybir as mybir
    import concourse.tile as tile
    from concourse import bacc

    f32 = mybir.dt.float32
    bf16 = mybir.dt.bfloat16
    nc = bacc.Bacc("TRN2", target_bir_lowering=False, debug=False)

    # per step 8 chunks of [64, 512]: row-groups 32j (K=24) x col-groups 64k.
    # ACT casts psA (chunks j=0,1) -> fp8, DVE casts psB (j=2,3) -> fp8; the
    # separate PSUM tiles keep the two casts parallel. fp8 halves ship bytes;
    # rounding is monotone so the true row/col argmax still wins or ties.
    f8 = mybir.dt.float8e4
    in0_d = nc.dram_tensor("in0", [steps // 2, 4, KSP, 2304], bf16, kind="ExternalInput")
    outa_d = nc.dram_tensor("outa", [steps // 2, 128, 2048], f8, kind="ExternalOutput")
    outb_d = nc.dram_tensor("outb", [steps // 2, 128, 2048], f8, kind="ExternalOutput")

    with tile.TileContext(nc) as tc, ExitStack() as ctx:
        io_pool = ctx.enter_context(tc.tile_pool(name="io", bufs=6))
        psum_pool = ctx.enter_context(tc.tile_pool(name="psum", bufs=2, space="PSUM"))
        stage_pool = ctx.enter_context(tc.tile_pool(name="stage", bufs=6))

        in_eng = {0: nc.sync, 1: nc.sync, 2: nc.gpsimd, 3: nc.scalar}
        t_in = None
        for s in range(steps):
            if s % 2 == 0:
                t_in = io_pool.tile([128, 2304], bf16)
                for j in range(4):
                    in_eng[j].dma_start(
                        t_in[32 * j:32 * j + KSP, :], in0_d[s // 2, j, :, :]
                    )
            # one [128, 512] PSUM tile per j: 4 tags x bufs=2 = all 8 banks;
            # each caster sees a 4-deep pipeline, hiding the MM latency
            pst = [psum_pool.tile([128, 512], f32, tag=f"ps{j}", name=f"ps{j}")
                   for j in range(4)]
            for k in range(2):
                for j in range(4):
                    o = (s % 2) * 1152 + k * 576
                    nc.tensor.matmul(
                        pst[j][64 * k:64 * k + 64, :],
                        t_in[32 * j:32 * j + KS, o:o + 64],
                        t_in[32 * j:32 * j + KS, o + 64:o + 576],
                        start=True,
                        stop=True,
                        tile_position=(32 * j, 64 * k),
                    )
            ph = s % 2
            sta = stage_pool.tile([128, 1024], f8, tag="sta")
            stb = stage_pool.tile([128, 1024], f8, tag="stb")
            nc.scalar.copy(sta[:, 0:512], pst[0][:])
            nc.scalar.copy(sta[:, 512:1024], pst[1][:])
            nc.vector.tensor_copy(stb[:, 0:512], pst[2][:])
            nc.vector.tensor_copy(stb[:, 512:1024], pst[3][:])
            nc.gpsimd.dma_start(outa_d[s // 2, :, ph * 1024:(ph + 1) * 1024], sta[:])
            nc.sync.dma_start(outb_d[s // 2, :, ph * 1024:(ph + 1) * 1024], stb[:])

    nc.compile()
    return nc


# ------------------------------------------------------------------ host: post
def _postprocess(preds, gts, normals, edges, results, metas, core_chunks, nreal):
    preds64 = preds.astype(np.float64)
    gts64 = gts.astype(np.float64)

    mins1 = np.empty((B, N), np.float64)
    mins2 = np.empty((B, N), np.float64)
    nearest_idx = np.empty((B, N), np.int64)

    # decode every core's output once: chunk i = s*8 + k*4 + j; outa holds
    # j=0,1 (k-halves in partitions), outb j=2,3
    devals = []
    for c in range(NCORES):
        va = np.asarray(results[c]["outa"], ml_dtypes.float8_e4m3).astype(np.float32)
        vb = np.asarray(results[c]["outb"], ml_dtypes.float8_e4m3).astype(np.float32)
        # [s2, 2(k), 64, 2(ph), 2(j2), 512] -> [(s,k), j2, 64, 512]
        va = va.reshape(-1, 2, TIL, 2, 2, CHW).transpose(0, 3, 1, 4, 2, 5)
        vb = vb.reshape(-1, 2, TIL, 2, 2, CHW).transpose(0, 3, 1, 4, 2, 5)
        va = va.reshape(-1, 2, 2, TIL, CHW)              # [s, k, j2, 64, 512]
        vb = vb.reshape(-1, 2, 2, TIL, CHW)
        v = np.concatenate([va, vb], axis=2).reshape(-1, TIL, CHW)
        devals.append(v[:nreal[c]])                      # [nch, 64, 512]

    for b in range(B):
        po, go = metas[b]['po'], metas[b]['go']
        # gather this batch's chunks from every core
        vals_all, cols_all, tile_all = [], [], []
        for c in range(NCORES):
            cc, colmap, tileof, batof = core_chunks[c]
            m = batof[:nreal[c]] == b
            vals_all.append(devals[c][m])
            cols_all.append(colmap[:nreal[c]][m])
            tile_all.append(tileof[:nreal[c]][m])
        vals = np.concatenate(vals_all)                  # [M, TIL, 512]
        cols = np.concatenate(cols_all)                  # [M, 512] sorted-p idx
        tils = np.concatenate(tile_all)                  # [M]

        # fp8 rounding is monotone: the true argmax always ties the quantized
        # max. Collect ALL tying candidates and resolve them exactly.
        G64, P64 = gts64[b], preds64[b]

        # ---- row path: per tile, max over its chunks' columns
        order = np.argsort(tils, kind='stable')
        vals_o, cols_o, tils_o = vals[order], cols[order], tils[order]
        bounds = np.searchsorted(tils_o, np.arange(NT + 1))
        for T in range(NT):
            i0, i1 = bounds[T], bounds[T + 1]
            v = vals_o[i0:i1]                            # [m, TIL, 512]
            flat = v.transpose(1, 0, 2).reshape(TIL, -1)
            mx = flat.max(1, keepdims=True)
            ti, pos = np.nonzero(flat == mx)             # tied candidates
            ci, cj = divmod(pos, CHW)
            srt_p = cols_o[i0:i1][ci, cj]
            t_orig = go[T * TIL + ti]
            p_orig = po[srt_p]
            d = ((G64[t_orig] - P64[p_orig]) ** 2).sum(-1)
            o3 = np.lexsort((d, ti))                     # per t: min d first
            tu, first = np.unique(ti[o3], return_index=True)
            sel = o3[first]
            rows = go[T * TIL + tu]
            mins2[b, rows] = d[sel]
            nearest_idx[b, rows] = p_orig[sel]

        # ---- col path: per sorted-p column, max over all (chunk, t)
        ncols = np.full(N, -np.inf, np.float32)
        np.maximum.at(ncols, cols.ravel(),
                      vals.max(1).ravel())               # fp8 col max
        cand_mask = vals == ncols[cols][:, None, :]      # [M, TIL, 512] ties
        mi, ti, cj = np.nonzero(cand_mask)
        srt_p = cols[mi, cj]
        srt_t = tils[mi] * TIL + ti
        d = ((G64[go[srt_t]] - P64[po[srt_p]]) ** 2).sum(-1)
        o2 = np.lexsort((d, srt_p))
        fc, first = np.unique(srt_p[o2], return_index=True)
        assert len(fc) == N, "column coverage hole"
        sel = o2[first]
        mins1[b, po[fc]] = d[sel]

    loss_1 = mins1.mean()
    loss_2 = mins2.mean()
    chamfer = loss_1 + loss_2

    e0 = edges[:, 0]
    e1 = edges[:, 1]
    edge_vectors = preds[:, e0, :] - preds[:, e1, :]
    edge_loss = (edge_vectors * edge_vectors).sum(axis=2).astype(np.float64).mean()

    normals_nearest = np.take_along_axis(normals, nearest_idx[:, :, None], axis=1)
    normals_edge = normals_nearest[:, e0, :]

    def l2n_dim1(v):
        n = np.sqrt((v * v).sum(axis=1, keepdims=True))
        return v / np.maximum(n, 1e-12)

    nn = l2n_dim1(normals_edge)
    nv = l2n_dim1(edge_vectors)
    cosines = np.abs((nn * nv).sum(axis=2))
    normal_cosine_loss = cosines.astype(np.float64).mean()

    return np.float32(
        30000.0 * chamfer + 240.0 * edge_loss + 200000.0 * normal_cosine_loss
    )


def kernel(preds, gts, normals, edges, _trace=False):
    from concourse.bass_utils import run_bass_kernel_spmd

    preds = np.asarray(preds, np.float32)
    gts = np.asarray(gts, np.float32)
    normals = np.asarray(normals, np.float32)
    edges = np.asarray(edges)

    metas, core_data, nreal, steps, in_maps = _prep(preds, gts)
    nc = _build_nc(steps)
    br = run_bass_kernel_spmd(nc, in_maps, list(range(NCORES)), trace=_trace)
    _LAST_RESULTS["bass_results"] = br
    return _postprocess(preds, gts, normals, edges, br.results,
                        metas, core_data, nreal)


# revision 51
# speedup vs baseline: 1.0863x; 1.0863x over previous
"""Chamfer + edge + normal-cosine combined loss on 8 Trainium2 cores.

Candidate-pruned distance scan. The host kd-sorts both point sets per batch,
computes bbox lower bounds LB(64-pt t-tile, 8-pt p-group) and per-point
achievable upper bounds on nearest distances, and keeps only (tile, group)
pairs that can contain a row-min (LB <= UB_tile) or a column-min
(LB <= UB_group) -- provably covering every exact row/column argmin (~10-20%
of all pairs survive). Kept groups are packed into 512-column chunks and
dealt evenly to the 8 cores (any core may process any batch's chunks).

The device is a pure streaming scanner. Per step, eight K=13 matmuls (2-way
bf16-split factors; tile_position row-groups 32j x col-groups 64k) compute
eight [64, 512] chunks of M = -P into four [128, 512] PSUM tiles; ACT casts
two of them to fp8 staging while DVE casts the other two in parallel (fp8
halves the ship bytes; rounding is monotone so the true row/col argmax always
wins or ties), and per-step DMAs on the gpsimd/sync queues ship the chunks to
DRAM while sync/gpsimd/scalar queues stream the next inputs in.

Host finish: fp8 row/col maxes per tile/column, ALL tying candidates
recomputed exactly in fp64 (restores ~1e-7 accuracy), then the tiny edge and
normal-cosine terms in numpy as in the reference.
"""

from contextlib import ExitStack

import ml_dtypes
import numpy as np

B = 4
N = 8192
NCORES = 8
TIL = 64             # t rows per tile (two tiles stack in 128 partitions)
GRP = 4              # fine p-group size for pruning
CHW = 512            # chunk width in columns
GPC = CHW // GRP     # 32 fine groups per chunk
NT = N // TIL        # 64 t-tiles per batch
NGR = N // GRP       # 512 fine groups per batch
KS = 13              # bf16 split rows (2-way split: ~1e-4 abs error on P,
                     # absorbed by the host's exact tie resolution)
KSP = 16             # split rows padded for DMA blocks
NNEAR = 5            # groups sampled for upper bounds

_LAST_RESULTS = {}


# ---------------------------------------------------------------- host: split
def _split3(x):
    h = x.astype(ml_dtypes.bfloat16)
    r1 = x - h.astype(np.float32)
    m = r1.astype(ml_dtypes.bfloat16)
    r2 = r1 - m.astype(np.float32)
    l = r2.astype(ml_dtypes.bfloat16)
    return h, m, l


def _build_split_rows(L, R):
    """L [5, X], R [5, Y] fp32 term rows -> bf16 [13, X], [13, Y].

    M = sum_k L[k] (outer) R[k] = 2<g,p> - |g|^2 - |p|^2 = -P; 2-way bf16
    split keeps products accurate to ~2^-16 relative (~1e-4 abs on P)."""
    outL, outR = [], []
    for c in range(3):
        Lh, Lm, _ = _split3(L[c])
        Rh, Rm, _ = _split3(R[c])
        for a, b in ((Lh, Rh), (Lh, Rm), (Lm, Rh)):
            outL.append(a)
            outR.append(b)
    Xh, Xm, _ = _split3(L[3])
    negone = R[3].astype(ml_dtypes.bfloat16)
    for a in (Xh, Xm):
        outL.append(a)
        outR.append(negone)
    Yh, Ym, _ = _split3(R[4])
    one = L[4].astype(ml_dtypes.bfloat16)
    for b in (Yh, Ym):
        outL.append(one)
        outR.append(b)
    return np.ascontiguousarray(np.stack(outL)), np.ascontiguousarray(np.stack(outR))


# -------------------------------------------------------------- host: pruning
def _kd_order(pts, leaf):
    """Balanced kd-tree order: median split on widest axis down to `leaf`."""
    out = []
    stack = [np.arange(len(pts))]
    while stack:
        ids = stack.pop()
        if len(ids) <= leaf:
            out.append(ids)
            continue
        p = pts[ids]
        ax = int((p.max(0) - p.min(0)).argmax())
        k = len(ids) // 2
        o = np.argpartition(p[:, ax], k)
        stack.append(ids[o[k:]])
        stack.append(ids[o[:k]])
    # stack order: first-pushed-last; rebuild in left-to-right order
    return np.concatenate(out)


def _point_ubs(A, Btiles, nnear):
    """For each point in A [n,3]: an achievable nearest-distance^2 upper bound,
    the min over all points of the `nnear` nearest B-tiles (by center)."""
    bc = Btiles.mean(1)
    d = ((A[:, None, :] - bc[None, :, :]) ** 2).sum(-1)
    near = np.argpartition(d, nnear, axis=1)[:, :nnear]
    ub = np.full(len(A), np.inf)
    for j in range(nnear):
        sel = near[:, j]
        for g in np.unique(sel):
            m = sel == g
            dd = ((A[m][:, None, :] - Btiles[g][None, :, :]) ** 2).sum(-1).min(1)
            ub[m] = np.minimum(ub[m], dd)
    return ub


def _prep_batch(preds_b, gts_b):
    """Returns sorted perms, per-tile candidate chunk lists and split tables."""
    po = _kd_order(preds_b, GRP)
    go = _kd_order(gts_b, TIL)
    Ps = preds_b[po].astype(np.float64)
    Gs = gts_b[go].astype(np.float64)

    Pt = Ps.reshape(NGR, GRP, 3)
    Gt = Gs.reshape(NT, TIL, 3)
    plo, phi = Pt.min(1), Pt.max(1)
    glo, ghi = Gt.min(1), Gt.max(1)
    d1 = np.maximum(0.0, plo[None, :, :] - ghi[:, None, :])
    d2 = np.maximum(0.0, glo[:, None, :] - phi[None, :, :])
    LB = (np.maximum(d1, d2) ** 2).sum(-1)           # [NT, NGR]

    ub_t = _point_ubs(Gs, Pt, NNEAR)
    UB_T = ub_t.reshape(NT, TIL).max(1)              # [NT]
    ub_p = _point_ubs(Ps, Gt, NNEAR)
    UB_G = ub_p.reshape(NGR, GRP).max(1)             # [NGR]

    keep = LB <= np.maximum(UB_T[:, None], UB_G[None, :]) * (1.0 + 1e-6) + 1e-12

    # chunk lists: per tile, its fine groups packed into CHW-wide chunks
    chunks = []                                      # (tile, group_ids[GPC])
    for T in range(NT):
        gl = np.nonzero(keep[T])[0]
        padded = ((len(gl) + GPC - 1) // GPC) * GPC
        gl = np.resize(gl, padded)      # cycles values to pad
        for c in range(len(gl) // GPC):
            chunks.append((T, gl[c * GPC:(c + 1) * GPC]))

    # split tables over sorted points
    xsq = (Gs * Gs).sum(-1).astype(np.float32)
    ysq = (Ps * Ps).sum(-1).astype(np.float32)
    L = np.empty((5, N), np.float32)
    L[0:3] = (2.0 * Gs.T).astype(np.float32)
    L[3] = xsq
    L[4] = 1.0
    R = np.empty((5, N), np.float32)
    R[0:3] = Ps.T.astype(np.float32)
    R[3] = -1.0
    R[4] = -ysq
    sL, sR = _build_split_rows(L, R)                 # [24, N] bf16 each
    return dict(po=po, go=go, chunks=chunks, sL=sL, sR=sR)


def _prep(preds, gts):
    metas = [_prep_batch(preds[b], gts[b]) for b in range(B)]
    # global load balance: flatten (batch, tile, groups) chunks, deal to cores
    allch = []
    for b in range(B):
        allch.extend((b,) + ch for ch in metas[b]['chunks'])
    per = (len(allch) + NCORES - 1) // NCORES
    raw = [allch[c * per:(c + 1) * per] for c in range(NCORES)]
    nreal = [len(cc) for cc in raw]
    # 8 chunks per step (4 row-groups x 2 col-groups); 2 steps per in-DMA
    steps = max((n + 7) // 8 for n in nreal)
    steps += steps % 2
    in_maps = []
    core_data = []
    for c in range(NCORES):
        cc = list(raw[c]) or [allch[0]]
        while len(cc) < steps * 8:
            cc.append(cc[-1])
        # packed row blocks: in0[s2, j] = [16, 2304] for j-block partitions
        # 32j+[0,16); cols ph*1152 + k*576 + [0:64 weights | 64:576 rhs]
        in0 = np.zeros((steps // 2, 4, KSP, 2304), ml_dtypes.bfloat16)
        colmap = np.empty((steps * 8, CHW), np.int32)
        tileof = np.empty(steps * 8, np.int32)
        batof = np.empty(steps * 8, np.int32)
        for i, (bb, T, gl) in enumerate(cc):
            sL, sR = metas[bb]['sL'], metas[bb]['sR']
            cols = (gl[:, None] * GRP + np.arange(GRP)[None, :]).ravel()
            s, slot = divmod(i, 8)
            k, j = divmod(slot, 4)
            s2, ph = divmod(s, 2)
            o = ph * 1152 + k * 576
            in0[s2, j, 0:KS, o:o + 64] = sL[:, T * TIL:(T + 1) * TIL]
            in0[s2, j, 0:KS, o + 64:o + 576] = sR[:, cols]
            colmap[i] = cols
            tileof[i] = T
            batof[i] = bb
        in_maps.append({"in0": in0})
        core_data.append((cc, colmap, tileof, batof))
    return metas, core_data, nreal, steps, in_maps


# ------------------------------------------------------------------- device
def _build_nc(steps):
    import concourse.mybir as mybir
    import concourse.tile as tile
    from concourse import bacc

    f32 = mybir.dt.float32
    bf16 = mybir.dt.bfloat16
    nc = bacc.Bacc("TRN2", target_bir_lowering=False, debug=False)

    # per step 8 chunks of [64, 512]: row-groups 32j (K=24) x col-groups 64k.
    # ACT casts psA (chunks j=0,1) -> fp8, DVE casts psB (j=2,3) -> fp8; the
    # separate PSUM tiles keep the two casts parallel. fp8 halves ship bytes;
    # rounding is monotone so the true row/col argmax still wins or ties.
    f8 = mybir.dt.float8e4
    in0_d = nc.dram_tensor("in0", [steps // 2, 4, KSP, 2304], bf16, kind="ExternalInput")
    outa_d = nc.dram_tensor("outa", [steps // 2, 128, 2048], f8, kind="ExternalOutput")
    outb_d = nc.dram_tensor("outb", [steps // 2, 128, 2048], f8, kind="ExternalOutput")

    with tile.TileContext(nc) as tc, ExitStack() as ctx:
        io_pool = ctx.enter_context(tc.tile_pool(name="io", bufs=4))
        psum_pool = ctx.enter_context(tc.tile_pool(name="psum", bufs=2, space="PSUM"))
        stage_pool = ctx.enter_context(tc.tile_pool(name="stage", bufs=4))

        in_eng = {0: nc.sync, 1: nc.sync, 2: nc.gpsimd, 3: nc.scalar}
        t_in = None
        for s in range(steps):
            if s % 2 == 0:
                t_in = io_pool.tile([128, 2304], bf16)
                for j in range(4):
                    in_eng[j].dma_start(
                        t_in[32 * j:32 * j + KSP, :], in0_d[s // 2, j, :, :]
                    )
            # one [128, 512] PSUM tile per j: 4 tags x bufs=2 = all 8 banks;
            # each caster sees a 4-deep pipeline, hiding the MM latency
            pst = [psum_pool.tile([128, 512], f32, tag=f"ps{j}", name=f"ps{j}")
                   for j in range(4)]
            for k in range(2):
                for j in range(4):
                    o = (s % 2) * 1152 + k * 576
                    nc.tensor.matmul(
                        pst[j][64 * k:64 * k + 64, :],
                        t_in[32 * j:32 * j + KS, o:o + 64],
                        t_in[32 * j:32 * j + KS, o + 64:o + 576],
                        start=True,
                        stop=True,
                        tile_position=(32 * j, 64 * k),
                    )
            ph = s % 2
            sta = stage_pool.tile([128, 1024], f8, tag="sta")
            stb = stage_pool.tile([128, 1024], f8, tag="stb")
            nc.scalar.copy(sta[:, 0:512], pst[0][:])
            nc.scalar.copy(sta[:, 512:1024], pst[1][:])
            nc.vector.tensor_copy(stb[:, 0:512], pst[2][:])
            nc.vector.tensor_copy(stb[:, 512:1024], pst[3][:])
            nc.gpsimd.dma_start(outa_d[s // 2, :, ph * 1024:(ph + 1) * 1024], sta[:])
            nc.sync.dma_start(outb_d[s // 2, :, ph * 1024:(ph + 1) * 1024], stb[:])

    nc.compile()
    return nc


# ------------------------------------------------------------------ host: post
def _postprocess(preds, gts, normals, edges, results, metas, core_chunks, nreal):
    preds64 = preds.astype(np.float64)
    gts64 = gts.astype(np.float64)

    mins1 = np.empty((B, N), np.float64)
    mins2 = np.empty((B, N), np.float64)
    nearest_idx = np.empty((B, N), np.int64)

    # decode every core's output once: chunk i = s*8 + k*4 + j; outa holds
    # j=0,1 (k-halves in partitions), outb j=2,3
    devals = []
    for c in range(NCORES):
        va = np.asarray(results[c]["outa"], ml_dtypes.float8_e4m3).astype(np.float32)
        vb = np.asarray(results[c]["outb"], ml_dtypes.float8_e4m3).astype(np.float32)
        # [s2, 2(k), 64, 2(ph), 2(j2), 512] -> [(s,k), j2, 64, 512]
        va = va.reshape(-1, 2, TIL, 2, 2, CHW).transpose(0, 3, 1, 4, 2, 5)
        vb = vb.reshape(-1, 2, TIL, 2, 2, CHW).transpose(0, 3, 1, 4, 2, 5)
        va = va.reshape(-1, 2, 2, TIL, CHW)              # [s, k, j2, 64, 512]
        vb = vb.reshape(-1, 2, 2, TIL, CHW)
        v = np.concatenate([va, vb], axis=2).reshape(-1, TIL, CHW)
        devals.append(v[:nreal[c]])                      # [nch, 64, 512]

    for b in range(B):
        po, go = metas[b]['po'], metas[b]['go']
        # gather this batch's chunks from every core
        vals_all, cols_all, tile_all = [], [], []
        for c in range(NCORES):
            cc, colmap, tileof, batof = core_chunks[c]
            m = batof[:nreal[c]] == b
            vals_all.append(devals[c][m])
            cols_all.append(colmap[:nreal[c]][m])
            tile_all.append(tileof[:nreal[c]][m])
        vals = np.concatenate(vals_all)                  # [M, TIL, 512]
        cols = np.concatenate(cols_all)                  # [M, 512] sorted-p idx
        tils = np.concatenate(tile_all)                  # [M]

        # fp8 rounding is monotone: the true argmax always ties the quantized
        # max. Collect ALL tying candidates and resolve them exactly.
        G64, P64 = gts64[b], preds64[b]

        # ---- row path: per tile, max over its chunks' columns
        order = np.argsort(tils, kind='stable')
        vals_o, cols_o, tils_o = vals[order], cols[order], tils[order]
        bounds = np.searchsorted(tils_o, np.arange(NT + 1))
        for T in range(NT):
            i0, i1 = bounds[T], bounds[T + 1]
            v = vals_o[i0:i1]                            # [m, TIL, 512]
            flat = v.transpose(1, 0, 2).reshape(TIL, -1)
            mx = flat.max(1, keepdims=True)
            ti, pos = np.nonzero(flat == mx)             # tied candidates
            ci, cj = divmod(pos, CHW)
            srt_p = cols_o[i0:i1][ci, cj]
            t_orig = go[T * TIL + ti]
            p_orig = po[srt_p]
            d = ((G64[t_orig] - P64[p_orig]) ** 2).sum(-1)
            o3 = np.lexsort((d, ti))                     # per t: min d first
            tu, first = np.unique(ti[o3], return_index=True)
            sel = o3[first]
            rows = go[T * TIL + tu]
            mins2[b, rows] = d[sel]
            nearest_idx[b, rows] = p_orig[sel]

        # ---- col path: per sorted-p column, max over all (chunk, t)
        ncols = np.full(N, -np.inf, np.float32)
        np.maximum.at(ncols, cols.ravel(),
                      vals.max(1).ravel())               # fp8 col max
        cand_mask = vals == ncols[cols][:, None, :]      # [M, TIL, 512] ties
        mi, ti, cj = np.nonzero(cand_mask)
        srt_p = cols[mi, cj]
        srt_t = tils[mi] * TIL + ti
        d = ((G64[go[srt_t]] - P64[po[srt_p]]) ** 2).sum(-1)
        o2 = np.lexsort((d, srt_p))
        fc, first = np.unique(srt_p[o2], return_index=True)
        assert len(fc) == N, "column coverage hole"
        sel = o2[first]
        mins1[b, po[fc]] = d[sel]

    loss_1 = mins1.mean()
    loss_2 = mins2.mean()
    chamfer = loss_1 + loss_2

    e0 = edges[:, 0]
    e1 = edges[:, 1]
    edge_vectors = preds[:, e0, :] - preds[:, e1, :]
    edge_loss = (edge_vectors * edge_vectors).sum(axis=2).astype(np.float64).mean()

    normals_nearest = np.take_along_axis(normals, nearest_idx[:, :, None], axis=1)
    normals_edge = normals_nearest[:, e0, :]

    def l2n_dim1(v):
        n = np.sqrt((v * v).sum(axis=1, keepdims=True))
        return v / np.maximum(n, 1e-12)

    nn = l2n_dim1(normals_edge)
    nv = l2n_dim1(edge_vectors)
    cosines = np.abs((nn * nv).sum(axis=2))
    normal_cosine_loss = cosines.astype(np.float64).mean()

    return np.float32(
        30000.0 * chamfer + 240.0 * edge_loss + 200000.0 * normal_cosine_loss
    )


def kernel(preds, gts, normals, edges, _trace=False):
    from concourse.bass_utils import run_bass_kernel_spmd

    preds = np.asarray(preds, np.float32)
    gts = np.asarray(gts, np.float32)
    normals = np.asarray(normals, np.float32)
    edges = np.asarray(edges)

    metas, core_data, nreal, steps, in_maps = _prep(preds, gts)
    nc = _build_nc(steps)
    br = run_bass_kernel_spmd(nc, in_maps, list(range(NCORES)), trace=_trace)
    _LAST_RESULTS["bass_results"] = br
    return _postprocess(preds, gts, normals, edges, br.results,
                        metas, core_data, nreal)


# revision 52
# speedup vs baseline: 1.1163x; 1.0276x over previous
"""Chamfer + edge + normal-cosine combined loss on 8 Trainium2 cores.

Candidate-pruned distance scan. The host kd-sorts both point sets per batch,
computes bbox lower bounds LB(64-pt t-tile, 8-pt p-group) and per-point
achievable upper bounds on nearest distances, and keeps only (tile, group)
pairs that can contain a row-min (LB <= UB_tile) or a column-min
(LB <= UB_group) -- provably covering every exact row/column argmin (~10-20%
of all pairs survive). Kept groups are packed into 512-column chunks and
dealt evenly to the 8 cores (any core may process any batch's chunks).

The device is a pure streaming scanner. Per step, eight K=13 matmuls (2-way
bf16-split factors; tile_position row-groups 32j x col-groups 64k) compute
eight [64, 512] chunks of M = -P into four [128, 512] PSUM tiles; ACT casts
two of them to fp8 staging while DVE casts the other two in parallel (fp8
halves the ship bytes; rounding is monotone so the true row/col argmax always
wins or ties), and per-step DMAs on the gpsimd/sync queues ship the chunks to
DRAM while sync/gpsimd/scalar queues stream the next inputs in.

Host finish: fp8 row/col maxes per tile/column, ALL tying candidates
recomputed exactly in fp64 (restores ~1e-7 accuracy), then the tiny edge and
normal-cosine terms in numpy as in the reference.
"""

from contextlib import ExitStack

import ml_dtypes
import numpy as np

B = 4
N = 8192
NCORES = 8
TIL = 64             # t rows per tile (two tiles stack in 128 partitions)
GRP = 4              # fine p-group size for pruning
CHW = 512            # chunk width in columns
GPC = CHW // GRP     # 32 fine groups per chunk
NT = N // TIL        # 64 t-tiles per batch
NGR = N // GRP       # 512 fine groups per batch
KS = 13              # bf16 split rows (2-way split: ~1e-4 abs error on P,
                     # absorbed by the host's exact tie resolution)
KSP = 16             # split rows padded for DMA blocks
NNEAR = 5            # groups sampled for upper bounds

_LAST_RESULTS = {}


# ---------------------------------------------------------------- host: split
def _split3(x):
    h = x.astype(ml_dtypes.bfloat16)
    r1 = x - h.astype(np.float32)
    m = r1.astype(ml_dtypes.bfloat16)
    r2 = r1 - m.astype(np.float32)
    l = r2.astype(ml_dtypes.bfloat16)
    return h, m, l


def _build_split_rows(L, R):
    """L [5, X], R [5, Y] fp32 term rows -> bf16 [13, X], [13, Y].

    M = sum_k L[k] (outer) R[k] = 2<g,p> - |g|^2 - |p|^2 = -P; 2-way bf16
    split keeps products accurate to ~2^-16 relative (~1e-4 abs on P)."""
    outL, outR = [], []
    for c in range(3):
        Lh, Lm, _ = _split3(L[c])
        Rh, Rm, _ = _split3(R[c])
        for a, b in ((Lh, Rh), (Lh, Rm), (Lm, Rh)):
            outL.append(a)
            outR.append(b)
    Xh, Xm, _ = _split3(L[3])
    negone = R[3].astype(ml_dtypes.bfloat16)
    for a in (Xh, Xm):
        outL.append(a)
        outR.append(negone)
    Yh, Ym, _ = _split3(R[4])
    one = L[4].astype(ml_dtypes.bfloat16)
    for b in (Yh, Ym):
        outL.append(one)
        outR.append(b)
    return np.ascontiguousarray(np.stack(outL)), np.ascontiguousarray(np.stack(outR))


# -------------------------------------------------------------- host: pruning
def _kd_order(pts, leaf):
    """Balanced kd-tree order: median split on widest axis down to `leaf`."""
    out = []
    stack = [np.arange(len(pts))]
    while stack:
        ids = stack.pop()
        if len(ids) <= leaf:
            out.append(ids)
            continue
        p = pts[ids]
        ax = int((p.max(0) - p.min(0)).argmax())
        k = len(ids) // 2
        o = np.argpartition(p[:, ax], k)
        stack.append(ids[o[k:]])
        stack.append(ids[o[:k]])
    # stack order: first-pushed-last; rebuild in left-to-right order
    return np.concatenate(out)


def _point_ubs(A, Btiles, nnear):
    """For each point in A [n,3]: an achievable nearest-distance^2 upper bound,
    the min over all points of the `nnear` nearest B-tiles (by center)."""
    bc = Btiles.mean(1)
    d = ((A[:, None, :] - bc[None, :, :]) ** 2).sum(-1)
    near = np.argpartition(d, nnear, axis=1)[:, :nnear]
    ub = np.full(len(A), np.inf)
    for j in range(nnear):
        sel = near[:, j]
        for g in np.unique(sel):
            m = sel == g
            dd = ((A[m][:, None, :] - Btiles[g][None, :, :]) ** 2).sum(-1).min(1)
            ub[m] = np.minimum(ub[m], dd)
    return ub


def _prep_batch(preds_b, gts_b):
    """Returns sorted perms, per-tile candidate chunk lists and split tables."""
    po = _kd_order(preds_b, GRP)
    go = _kd_order(gts_b, TIL)
    Ps = preds_b[po].astype(np.float64)
    Gs = gts_b[go].astype(np.float64)

    Pt = Ps.reshape(NGR, GRP, 3)
    Gt = Gs.reshape(NT, TIL, 3)
    plo, phi = Pt.min(1), Pt.max(1)
    glo, ghi = Gt.min(1), Gt.max(1)
    d1 = np.maximum(0.0, plo[None, :, :] - ghi[:, None, :])
    d2 = np.maximum(0.0, glo[:, None, :] - phi[None, :, :])
    LB = (np.maximum(d1, d2) ** 2).sum(-1)           # [NT, NGR]

    ub_t = _point_ubs(Gs, Pt, NNEAR)
    UB_T = ub_t.reshape(NT, TIL).max(1)              # [NT]
    ub_p = _point_ubs(Ps, Gt, NNEAR)
    UB_G = ub_p.reshape(NGR, GRP).max(1)             # [NGR]

    keep = LB <= np.maximum(UB_T[:, None], UB_G[None, :]) * (1.0 + 1e-6) + 1e-12

    # chunk lists: per tile, its fine groups packed into CHW-wide chunks
    chunks = []                                      # (tile, group_ids[GPC])
    for T in range(NT):
        gl = np.nonzero(keep[T])[0]
        padded = ((len(gl) + GPC - 1) // GPC) * GPC
        gl = np.resize(gl, padded)      # cycles values to pad
        for c in range(len(gl) // GPC):
            chunks.append((T, gl[c * GPC:(c + 1) * GPC]))

    # split tables over sorted points
    xsq = (Gs * Gs).sum(-1).astype(np.float32)
    ysq = (Ps * Ps).sum(-1).astype(np.float32)
    L = np.empty((5, N), np.float32)
    L[0:3] = (2.0 * Gs.T).astype(np.float32)
    L[3] = xsq
    L[4] = 1.0
    R = np.empty((5, N), np.float32)
    R[0:3] = Ps.T.astype(np.float32)
    R[3] = -1.0
    R[4] = -ysq
    sL, sR = _build_split_rows(L, R)                 # [24, N] bf16 each
    return dict(po=po, go=go, chunks=chunks, sL=sL, sR=sR)


def _prep(preds, gts):
    metas = [_prep_batch(preds[b], gts[b]) for b in range(B)]
    # global load balance: flatten (batch, tile, groups) chunks, deal to cores
    allch = []
    for b in range(B):
        allch.extend((b,) + ch for ch in metas[b]['chunks'])
    per = (len(allch) + NCORES - 1) // NCORES
    raw = [allch[c * per:(c + 1) * per] for c in range(NCORES)]
    nreal = [len(cc) for cc in raw]
    # 8 chunks per step (4 row-groups x 2 col-groups); 2 steps per in-DMA
    steps = max((n + 7) // 8 for n in nreal)
    steps += steps % 2
    in_maps = []
    core_data = []
    for c in range(NCORES):
        cc = list(raw[c]) or [allch[0]]
        while len(cc) < steps * 8:
            cc.append(cc[-1])
        # packed row blocks: in0[s2, j] = [16, 2304] for j-block partitions
        # 32j+[0,16); cols ph*1152 + k*576 + [0:64 weights | 64:576 rhs]
        in0 = np.zeros((steps // 2, 4, KSP, 2304), ml_dtypes.bfloat16)
        colmap = np.empty((steps * 8, CHW), np.int32)
        tileof = np.empty(steps * 8, np.int32)
        batof = np.empty(steps * 8, np.int32)
        for i, (bb, T, gl) in enumerate(cc):
            sL, sR = metas[bb]['sL'], metas[bb]['sR']
            cols = (gl[:, None] * GRP + np.arange(GRP)[None, :]).ravel()
            s, slot = divmod(i, 8)
            k, j = divmod(slot, 4)
            s2, ph = divmod(s, 2)
            o = ph * 1152 + k * 576
            in0[s2, j, 0:KS, o:o + 64] = sL[:, T * TIL:(T + 1) * TIL]
            in0[s2, j, 0:KS, o + 64:o + 576] = sR[:, cols]
            colmap[i] = cols
            tileof[i] = T
            batof[i] = bb
        in_maps.append({"in0": in0})
        core_data.append((cc, colmap, tileof, batof))
    return metas, core_data, nreal, steps, in_maps


# ------------------------------------------------------------------- device
def _build_nc(steps):
    import concourse.mybir as mybir
    import concourse.tile as tile
    from concourse import bacc

    f32 = mybir.dt.float32
    bf16 = mybir.dt.bfloat16
    nc = bacc.Bacc("TRN2", target_bir_lowering=False, debug=False)

    # per step 8 chunks of [64, 512]: row-groups 32j (K=24) x col-groups 64k.
    # ACT casts psA (chunks j=0,1) -> fp8, DVE casts psB (j=2,3) -> fp8; the
    # separate PSUM tiles keep the two casts parallel. fp8 halves ship bytes;
    # rounding is monotone so the true row/col argmax still wins or ties.
    f8 = mybir.dt.float8e4
    in0_d = nc.dram_tensor("in0", [steps // 2, 4, KSP, 2304], bf16, kind="ExternalInput")
    outa_d = nc.dram_tensor("outa", [steps // 2, 128, 2048], f8, kind="ExternalOutput")
    outb_d = nc.dram_tensor("outb", [steps // 2, 128, 2048], f8, kind="ExternalOutput")

    with tile.TileContext(nc) as tc, ExitStack() as ctx:
        io_pool = ctx.enter_context(tc.tile_pool(name="io", bufs=4))
        psum_pool = ctx.enter_context(tc.tile_pool(name="psum", bufs=2, space="PSUM"))
        stage_pool = ctx.enter_context(tc.tile_pool(name="stage", bufs=4))

        in_eng = {0: nc.sync, 1: nc.sync, 2: nc.gpsimd, 3: nc.scalar}
        t_in = None
        for s in range(steps):
            if s % 2 == 0:
                t_in = io_pool.tile([128, 2304], bf16)
                for j in range(4):
                    in_eng[j].dma_start(
                        t_in[32 * j:32 * j + KSP, :], in0_d[s // 2, j, :, :]
                    )
            # ACT gets one [128, 1024] PSUM tile (j=0,1: single wide cast on
            # the busiest engine); DVE keeps two [128, 512] tiles (4-deep)
            psa = psum_pool.tile([128, 1024], f32, tag="psa", name="psa")
            psb = [psum_pool.tile([128, 512], f32, tag=f"psb{j}", name=f"psb{j}")
                   for j in range(2)]
            for k in range(2):
                for j in range(4):
                    o = (s % 2) * 1152 + k * 576
                    out_ap = (psa[64 * k:64 * k + 64, j * 512:(j + 1) * 512]
                              if j < 2 else psb[j - 2][64 * k:64 * k + 64, :])
                    nc.tensor.matmul(
                        out_ap,
                        t_in[32 * j:32 * j + KS, o:o + 64],
                        t_in[32 * j:32 * j + KS, o + 64:o + 576],
                        start=True,
                        stop=True,
                        tile_position=(32 * j, 64 * k),
                    )
            ph = s % 2
            sta = stage_pool.tile([128, 1024], f8, tag="sta")
            stb = stage_pool.tile([128, 1024], f8, tag="stb")
            nc.scalar.copy(sta[:], psa[:])
            nc.vector.tensor_copy(stb[:, 0:512], psb[0][:])
            nc.vector.tensor_copy(stb[:, 512:1024], psb[1][:])
            nc.gpsimd.dma_start(outa_d[s // 2, :, ph * 1024:(ph + 1) * 1024], sta[:])
            nc.sync.dma_start(outb_d[s // 2, :, ph * 1024:(ph + 1) * 1024], stb[:])

    nc.compile()
    return nc


# ------------------------------------------------------------------ host: post
def _postprocess(preds, gts, normals, edges, results, metas, core_chunks, nreal):
    preds64 = preds.astype(np.float64)
    gts64 = gts.astype(np.float64)

    mins1 = np.empty((B, N), np.float64)
    mins2 = np.empty((B, N), np.float64)
    nearest_idx = np.empty((B, N), np.int64)

    # decode every core's output once: chunk i = s*8 + k*4 + j; outa holds
    # j=0,1 (k-halves in partitions), outb j=2,3
    devals = []
    for c in range(NCORES):
        va = np.asarray(results[c]["outa"], ml_dtypes.float8_e4m3).astype(np.float32)
        vb = np.asarray(results[c]["outb"], ml_dtypes.float8_e4m3).astype(np.float32)
        # [s2, 2(k), 64, 2(ph), 2(j2), 512] -> [(s,k), j2, 64, 512]
        va = va.reshape(-1, 2, TIL, 2, 2, CHW).transpose(0, 3, 1, 4, 2, 5)
        vb = vb.reshape(-1, 2, TIL, 2, 2, CHW).transpose(0, 3, 1, 4, 2, 5)
        va = va.reshape(-1, 2, 2, TIL, CHW)              # [s, k, j2, 64, 512]
        vb = vb.reshape(-1, 2, 2, TIL, CHW)
        v = np.concatenate([va, vb], axis=2).reshape(-1, TIL, CHW)
        devals.append(v[:nreal[c]])                      # [nch, 64, 512]

    for b in range(B):
        po, go = metas[b]['po'], metas[b]['go']
        # gather this batch's chunks from every core
        vals_all, cols_all, tile_all = [], [], []
        for c in range(NCORES):
            cc, colmap, tileof, batof = core_chunks[c]
            m = batof[:nreal[c]] == b
            vals_all.append(devals[c][m])
            cols_all.append(colmap[:nreal[c]][m])
            tile_all.append(tileof[:nreal[c]][m])
        vals = np.concatenate(vals_all)                  # [M, TIL, 512]
        cols = np.concatenate(cols_all)                  # [M, 512] sorted-p idx
        tils = np.concatenate(tile_all)                  # [M]

        # fp8 rounding is monotone: the true argmax always ties the quantized
        # max. Collect ALL tying candidates and resolve them exactly.
        G64, P64 = gts64[b], preds64[b]

        # ---- row path: per tile, max over its chunks' columns
        order = np.argsort(tils, kind='stable')
        vals_o, cols_o, tils_o = vals[order], cols[order], tils[order]
        bounds = np.searchsorted(tils_o, np.arange(NT + 1))
        for T in range(NT):
            i0, i1 = bounds[T], bounds[T + 1]
            v = vals_o[i0:i1]                            # [m, TIL, 512]
            flat = v.transpose(1, 0, 2).reshape(TIL, -1)
            mx = flat.max(1, keepdims=True)
            ti, pos = np.nonzero(flat == mx)             # tied candidates
            ci, cj = divmod(pos, CHW)
            srt_p = cols_o[i0:i1][ci, cj]
            t_orig = go[T * TIL + ti]
            p_orig = po[srt_p]
            d = ((G64[t_orig] - P64[p_orig]) ** 2).sum(-1)
            o3 = np.lexsort((d, ti))                     # per t: min d first
            tu, first = np.unique(ti[o3], return_index=True)
            sel = o3[first]
            rows = go[T * TIL + tu]
            mins2[b, rows] = d[sel]
            nearest_idx[b, rows] = p_orig[sel]

        # ---- col path: per sorted-p column, max over all (chunk, t)
        ncols = np.full(N, -np.inf, np.float32)
        np.maximum.at(ncols, cols.ravel(),
                      vals.max(1).ravel())               # fp8 col max
        cand_mask = vals == ncols[cols][:, None, :]      # [M, TIL, 512] ties
        mi, ti, cj = np.nonzero(cand_mask)
        srt_p = cols[mi, cj]
        srt_t = tils[mi] * TIL + ti
        d = ((G64[go[srt_t]] - P64[po[srt_p]]) ** 2).sum(-1)
        o2 = np.lexsort((d, srt_p))
        fc, first = np.unique(srt_p[o2], return_index=True)
        assert len(fc) == N, "column coverage hole"
        sel = o2[first]
        mins1[b, po[fc]] = d[sel]

    loss_1 = mins1.mean()
    loss_2 = mins2.mean()
    chamfer = loss_1 + loss_2

    e0 = edges[:, 0]
    e1 = edges[:, 1]
    edge_vectors = preds[:, e0, :] - preds[:, e1, :]
    edge_loss = (edge_vectors * edge_vectors).sum(axis=2).astype(np.float64).mean()

    normals_nearest = np.take_along_axis(normals, nearest_idx[:, :, None], axis=1)
    normals_edge = normals_nearest[:, e0, :]

    def l2n_dim1(v):
        n = np.sqrt((v * v).sum(axis=1, keepdims=True))
        return v / np.maximum(n, 1e-12)

    nn = l2n_dim1(normals_edge)
    nv = l2n_dim1(edge_vectors)
    cosines = np.abs((nn * nv).sum(axis=2))
    normal_cosine_loss = cosines.astype(np.float64).mean()

    return np.float32(
        30000.0 * chamfer + 240.0 * edge_loss + 200000.0 * normal_cosine_loss
    )


def kernel(preds, gts, normals, edges, _trace=False):
    from concourse.bass_utils import run_bass_kernel_spmd

    preds = np.asarray(preds, np.float32)
    gts = np.asarray(gts, np.float32)
    normals = np.asarray(normals, np.float32)
    edges = np.asarray(edges)

    metas, core_data, nreal, steps, in_maps = _prep(preds, gts)
    nc = _build_nc(steps)
    br = run_bass_kernel_spmd(nc, in_maps, list(range(NCORES)), trace=_trace)
    _LAST_RESULTS["bass_results"] = br
    return _postprocess(preds, gts, normals, edges, br.results,
                        metas, core_data, nreal)
